# revision 43
# baseline (speedup 1.0000x reference)
"""Multi-head attention (B=2, T=2048, D=1024, H=16) on 8 TRN2 NeuronCores.

Sharding: 2D (batch x head-group). Core c handles batch b = c // 4 and head
group hg = c % 4 (4 heads = 256 channels of the projected dim).

Design (the ScalarE exp stream is the pacer; everything else hides under
it -- its 128 ACTIVATEs are ~143us of the ~221us span):
  - The exp stream starts ~24us in: xq/xk are DMA'd in 512-column
    super-blocks in dependency-critical order; Q tile 0 + K super-block 0
    are projected first, so scores+exp start while the rest of K/Q/V still
    loads. K tb1-3 and all 16 V-proj chunks are fillers inside block (0,0),
    each issued before the PV/score matmul that consumes it (issue order
    IS dependency order for mutable tiles -- a read issued before a write
    legally reads the old contents).
  - V is projected directly into [t, j] layout (xv chunk as the stationary
    operand) -- no PE transpose pass.  V-proj, Q1-3 proj, normalization and
    the output projection are all PE fillers woven into the ACT-paced
    attention blocks.  PV matmuls may lag exp by many kt tiles (deep u pool).
  - Softmax denominators ride the PV matmul as a 65th row (ones column in
    V).  Reciprocals: the 4 denom rows of a qt round are DMA-gathered into a
    [128, 16] tile (partition-parallel), one cheap DVE reciprocal, and
    DMA-scattered back to [1, 512] rows for the broadcast matmul.  This
    replaces 53us of single-partition DVE reciprocal with ~1us.
  - K/Q biases ride the PSUM->SBUF evacuation as per-partition
    tensor_scalar adds; V bias rides the V-proj accumulation as a ones
    matmul.
  - PSUM: scores 2 bufs x 2 banks, PV accumulators 2 x 1 bank, fillers
    2 x 1 bank = 8 banks exactly.

Output partials are written bf16 (halves output DMA; well inside the
error budget). Host sums the 4 head-group partials per batch in fp32,
transposes, adds bo. All shapes are hardcoded. kernel() takes full
inputs, returns [2, 2048, 1024] fp32.
"""

import numpy as np
import ml_dtypes

import concourse.bass as bass
import concourse.bacc as bacc
import concourse.mybir as mybir
import concourse.tile as tile
from concourse.bass_utils import run_bass_kernel_spmd

B, T, D, H, Hd = 2, 2048, 1024, 16, 64
HPC = 4          # heads per core
W = HPC * Hd     # 256 projected channels per core
SCALE = Hd ** -0.5
N_CORES = 8
NTB = 4          # 512-col super-blocks of T

BF16 = mybir.dt.bfloat16
F32 = mybir.dt.float32
bf16 = ml_dtypes.bfloat16


def build_nc():
    nc = bacc.Bacc("TRN2", target_bir_lowering=False, debug=False)

    # x tensors swizzled host-side to [128, tb, c, 512] (tb-major contiguous)
    xq = nc.dram_tensor("xq", [128, NTB, 8, 512], BF16, kind="ExternalInput").ap()
    xk = nc.dram_tensor("xk", [128, NTB, 8, 512], BF16, kind="ExternalInput").ap()
    xv = nc.dram_tensor("xv", [128, NTB, 8, 512], BF16, kind="ExternalInput").ap()
    # weights host-preswizzled to [128, chunk, cols] DMA-contiguous layout
    wq = nc.dram_tensor("wq", [128, 8 * W], BF16, kind="ExternalInput").ap()
    wk = nc.dram_tensor("wk", [128, 8 * W], BF16, kind="ExternalInput").ap()
    wv = nc.dram_tensor("wv", [128, 8 * W], BF16, kind="ExternalInput").ap()
    wo = nc.dram_tensor("wo", [128, 2 * D], BF16, kind="ExternalInput").ap()
    # q/k biases as per-partition columns [128, 2]; v bias as a row [1, W]
    bqc = nc.dram_tensor("bqc", [128, 2], F32, kind="ExternalInput").ap()
    bkc = nc.dram_tensor("bkc", [128, 2], F32, kind="ExternalInput").ap()
    bv = nc.dram_tensor("bv", [1, W], BF16, kind="ExternalInput").ap()
    out = nc.dram_tensor("out", [D, T], BF16, kind="ExternalOutput").ap()

    Exp = mybir.ActivationFunctionType.Exp

    with tile.TileContext(nc) as tc:
        with (
            tc.tile_pool(name="persist", bufs=1) as persist,
            tc.tile_pool(name="xkpool", bufs=4) as xkpool,
            tc.tile_pool(name="xqpool", bufs=2) as xqpool,
            tc.tile_pool(name="upool", bufs=16) as upool,
            tc.tile_pool(name="opool", bufs=4) as opool,
            tc.tile_pool(name="rpool", bufs=4) as rpool,
            tc.tile_pool(name="psB", bufs=1, space="PSUM") as psB,
        ):
            # ---- constants ----
            onescol = persist.tile([1, 128], BF16, tag="onescol")
            nc.vector.memset(onescol, 1.0)
            bc1 = persist.tile([1, 64], BF16, tag="bc1")
            nc.vector.memset(bc1, 1.0)
            warm = persist.tile([1, 8], F32, tag="warm")
            nc.vector.memset(warm, 0.0)
            # preload the exp table set during the initial DMA wait
            warm2 = persist.tile([1, 8], BF16, tag="warm2")
            nc.scalar.activation(warm2, warm, Exp)

            # ---- weight/bias tiles (DMAs issued below in critical order) ----
            wq_sb = persist.tile([128, 8, W], BF16, tag="wq")
            bqc_sb = persist.tile([128, 2], F32, tag="bqc")
            wk_sb = persist.tile([128, 8, W], BF16, tag="wk")
            bkc_sb = persist.tile([128, 2], F32, tag="bkc")
            wv_sb = persist.tile([128, 8, W], BF16, tag="wv")
            bv_sb = persist.tile([1, W], BF16, tag="bv")

            # ---- persistent activations ----
            qt_sb = persist.tile([128, 2, T], BF16, tag="qt")   # QT [j, t]
            kt_sb = persist.tile([128, 2, T], BF16, tag="kt")   # KT [j, t]
            xv_sb = persist.tile([128, NTB, 8, 512], BF16, tag="xv")
            vaug_sb = persist.tile([128, 16, HPC, Hd + 1], BF16, tag="vaug")
            nc.vector.memset(vaug_sb[:, :, :, 64:65], 1.0)
            otn_sb = persist.tile([128, 2, T], BF16, tag="otn")  # normalized O.T
            # raw [O.T ; denom] slots: (qt%2)*4 + pr*2 + m
            oraw_sb = persist.tile([65, 8, 512], F32, tag="oraw")
            wo_sb = persist.tile([128, 2, D], BF16, tag="wo")
            # denominator gather/recip/scatter staging (double-buffered by qt parity)
            dT_sb = persist.tile([128, 2, 2, 8], F32, tag="dT")
            rT_sb = persist.tile([128, 2, 2, 8], BF16, tag="rT")
            rrow_sb = persist.tile([1, 2, 4, 512], BF16, tag="rrow")

            # ---- input DMAs, in exp-start critical-path order ----
            xk_tiles = [
                xkpool.tile([128, 8, 512], BF16, tag="xk", name=f"xk{tb}")
                for tb in range(NTB)]
            xq_tiles = [
                xqpool.tile([128, 8, 512], BF16, tag="xq", name=f"xq{tb}")
                for tb in range(NTB)]
            nc.sync.dma_start(out=wq_sb, in_=wq.rearrange("p (c j) -> p c j", j=W))
            nc.sync.dma_start(out=xq_tiles[0], in_=xq[:, 0])
            nc.sync.dma_start(out=wk_sb, in_=wk.rearrange("p (c j) -> p c j", j=W))
            nc.sync.dma_start(out=xk_tiles[0], in_=xk[:, 0])
            nc.sync.dma_start(out=bqc_sb, in_=bqc)
            nc.sync.dma_start(out=bkc_sb, in_=bkc)
            nc.sync.dma_start(out=wv_sb, in_=wv.rearrange("p (c j) -> p c j", j=W))
            nc.sync.dma_start(out=bv_sb, in_=bv)
            nc.sync.dma_start(out=xv_sb[:, 0], in_=xv[:, 0])
            nc.sync.dma_start(out=xk_tiles[1], in_=xk[:, 1])
            nc.sync.dma_start(out=xv_sb[:, 1], in_=xv[:, 1])
            nc.sync.dma_start(out=xk_tiles[2], in_=xk[:, 2])
            nc.sync.dma_start(out=xv_sb[:, 2], in_=xv[:, 2])
            nc.sync.dma_start(out=xk_tiles[3], in_=xk[:, 3])
            nc.sync.dma_start(out=xv_sb[:, 3], in_=xv[:, 3])
            for tb in range(1, NTB):
                nc.sync.dma_start(out=xq_tiles[tb], in_=xq[:, tb])
            nc.sync.dma_start(out=wo_sb, in_=wo.rearrange("p (c e) -> p c e", e=D))

            # ================= projections =================
            def q_proj(qt):
                """Project Q for 512-col tile qt (filler; 2 pieces)."""
                pieces = []
                for jt in range(2):
                    def run(jt=jt):
                        ps = psB.tile([128, 512], F32, tag="f", bufs=2,
                                      name=f"qps{qt}_{jt}")
                        for c in range(8):
                            nc.tensor.matmul(
                                ps, lhsT=wq_sb[:, c, jt * 128:(jt + 1) * 128],
                                rhs=xq_tiles[qt][:, c, :],
                                start=(c == 0), stop=(c == 7))
                        nc.vector.tensor_scalar_add(
                            qt_sb[:, jt, qt * 512:(qt + 1) * 512], ps,
                            bqc_sb[:, jt:jt + 1])
                    pieces.append(run)
                return pieces

            def k_proj_half(tb, jt):
                """Project one jt-half of K super-block tb (f pool, so
                in-block K fillers never starve the score double-buffer)."""
                ps = psB.tile([128, 512], F32, tag="f", bufs=2,
                              name=f"kps{tb}_{jt}")
                for c in range(8):
                    nc.tensor.matmul(
                        ps,
                        lhsT=wk_sb[:, c, jt * 128:(jt + 1) * 128],
                        rhs=xk_tiles[tb][:, c, :],
                        start=(c == 0), stop=(c == 7))
                nc.vector.tensor_scalar_add(
                    kt_sb[:, jt, tb * 512:(tb + 1) * 512], ps,
                    bkc_sb[:, jt:jt + 1])

            def k_proj(tb):
                for jt in range(2):
                    k_proj_half(tb, jt)

            def v_chunk(tc_):
                """Project V for 128-col t-chunk tc_ into vaug (filler)."""
                tb, sub = divmod(tc_, 4)

                def run():
                    vps = psB.tile([128, 256], F32, tag="f", bufs=2,
                                   name=f"vps{tc_}")
                    for c in range(8):
                        nc.tensor.matmul(
                            vps,
                            lhsT=xv_sb[:, tb, c, sub * 128:(sub + 1) * 128],
                            rhs=wv_sb[:, c, :],
                            start=(c == 0), stop=False)
                    nc.tensor.matmul(vps, lhsT=onescol, rhs=bv_sb,
                                     start=False, stop=True)
                    nc.vector.tensor_copy(
                        vaug_sb[:, tc_, :, 0:64],
                        vps.rearrange("p (h d) -> p h d", h=HPC))
                return run

            # Prefix: Q0 + K tb0 only -- the first scores need just these.
            # K tb1-3 are woven into block (0,0) as fillers.
            for p in q_proj(0):
                p()
            k_proj(0)

            # ============ attention blocks with woven fillers ============
            def attn_block(pr, qt, fillers=()):
                fillers = dict(fillers)
                qsl = slice(qt * 512, (qt + 1) * 512)
                o_psA = psB.tile([65, 512], F32, tag="oA", bufs=1, name="o_psA")
                o_psB = psB.tile([65, 512], F32, tag="oB", bufs=1, name="o_psB")
                us = []
                for kt in range(17):
                    if kt < 16:
                        s_big = psB.tile([128, 2, 512], F32, tag="s",
                                         bufs=2, name="s_big")
                        for m in range(2):
                            po = 64 * m
                            nc.tensor.matmul(
                                s_big[:, m, :],
                                lhsT=kt_sb[po:po + 64, pr,
                                           kt * 128:(kt + 1) * 128],
                                rhs=qt_sb[po:po + 64, pr, qsl],
                                start=True, stop=True)
                        u_big = upool.tile([128, 2, 512], BF16, tag="u",
                                           name="u_big")
                        nc.scalar.activation(u_big, s_big, Exp, scale=SCALE)
                        us.append(u_big)
                    if kt >= 1:
                        for m, o_ps in ((0, o_psA), (1, o_psB)):
                            h = 2 * pr + m
                            nc.tensor.matmul(
                                o_ps,
                                lhsT=vaug_sb[:, kt - 1, h, :],
                                rhs=us[kt - 1][:, m, :],
                                start=(kt == 1), stop=(kt == 16))
                    if kt in fillers:
                        for fn in fillers.pop(kt):
                            fn()
                for fns in fillers.values():
                    for fn in fns:
                        fn()
                # stage raw [O.T ; denom] to SBUF; denom rows first so
                # the reciprocal gather can start before the body copies
                for m, o_ps in ((0, o_psA), (1, o_psB)):
                    s = (qt % 2) * 4 + pr * 2 + m
                    nc.vector.tensor_copy(oraw_sb[64:65, s, :],
                                          o_ps[64:65, :])
                for m, o_ps in ((0, o_psA), (1, o_psB)):
                    s = (qt % 2) * 4 + pr * 2 + m
                    nc.vector.tensor_copy(oraw_sb[0:64, s, :], o_ps[0:64, :])

            def recip_block(pr, qt):
                """Gather one block's 2 denom rows -> [128,8], recip,
                scatter back (issued right after each block's oraw evac)."""
                par = qt % 2
                base = par * 4 + pr * 2
                nc.sync.dma_start(out=dT_sb[:, par, pr, :],
                                  in_=oraw_sb[64:65, base:base + 2, :])
                with nc.allow_low_precision(
                        reason="1/denom bf16; ample for softmax"):
                    nc.vector.reciprocal(rT_sb[:, par, pr, :],
                                         dT_sb[:, par, pr, :])
                nc.sync.dma_start(out=rrow_sb[:, par, pr * 2:pr * 2 + 2, :],
                                  in_=rT_sb[:, par, pr, :])

            def norm_pieces(qt):
                """Normalize O.T for q tile qt: 4 filler closures."""
                qsl = slice(qt * 512, (qt + 1) * 512)
                par = qt % 2

                def piece(pr, m):
                    def run():
                        s = par * 4 + pr * 2 + m
                        rb_ps = psB.tile([64, 512], F32, tag="f", bufs=2,
                                         name="rb_ps")
                        nc.tensor.matmul(
                            rb_ps, lhsT=bc1,
                            rhs=rrow_sb[0:1, par, pr * 2 + m, :],
                            start=True, stop=True)
                        if m == 0:
                            nc.vector.tensor_mul(
                                otn_sb[0:64, pr, qsl],
                                oraw_sb[0:64, s, :], rb_ps)
                        else:
                            otnB = rpool.tile([64, 512], BF16, tag="otnB",
                                              name="otnB")
                            nc.vector.tensor_mul(
                                otnB, oraw_sb[0:64, s, :], rb_ps)
                            nc.sync.dma_start(
                                out=otn_sb[64:128, pr, qsl], in_=otnB)
                    return run
                return [piece(0, 0), piece(0, 1), piece(1, 0), piece(1, 1)]

            def proj_pieces(qt, evac_scalar=False):
                """Output projection for q tile qt: 8 filler closures."""
                qsl = slice(qt * 512, (qt + 1) * 512)

                def piece(et):
                    def run():
                        # tail pieces alternate psum pool + evac engine so
                        # neither the banks nor one copy engine serialize
                        tag = "s" if (evac_scalar and et % 2) else "f"
                        e_ps = psB.tile([128, 512], F32, tag=tag, bufs=2,
                                        name="e_ps")
                        for jc in range(2):
                            nc.tensor.matmul(
                                e_ps,
                                lhsT=wo_sb[:, jc, et * 128:(et + 1) * 128],
                                rhs=otn_sb[:, jc, qsl],
                                start=(jc == 0), stop=(jc == 1))
                        stg = opool.tile([128, 512], BF16, tag="ostg",
                                         name="stg")
                        if evac_scalar and et % 2:
                            nc.scalar.copy(stg, e_ps)
                        else:
                            nc.vector.tensor_copy(stg, e_ps)
                        nc.sync.dma_start(
                            out=out[et * 128:(et + 1) * 128, qsl], in_=stg)
                    return run
                return [piece(et) for et in range(8)]

            # ---- main schedule ----
            for qt in range(4):
                if qt == 0:
                    # Block (0,0) fillers: K tb1-3 (tbJ before the scores of
                    # kt=4J, i.e. slot <= 4J-1) and V-proj chunk i at slot
                    # <= i (always ahead of the PV that consumes it).
                    f0 = {i: [v_chunk(i)] for i in range(16)}
                    f0[1].append(lambda: k_proj_half(1, 0))
                    f0[2].append(lambda: k_proj_half(1, 1))
                    f0[5].append(lambda: k_proj_half(2, 0))
                    f0[6].append(lambda: k_proj_half(2, 1))
                    f0[9].append(lambda: k_proj_half(3, 0))
                    f0[10].append(lambda: k_proj_half(3, 1))
                    f0 = list(f0.items())
                    qp = q_proj(1)
                    f1 = [(4, [qp[0]]), (10, [qp[1]])]
                else:
                    nps = norm_pieces(qt - 1)
                    f0 = [(5, [nps[0]]), (7, [nps[1]]), (9, [nps[2]]),
                          (11, [nps[3]])]
                    pps = proj_pieces(qt - 1)
                    f1 = [(s, [p]) for s, p in
                          zip([1, 3, 5, 7, 9, 11, 13, 15], pps)]
                    if qt < 3:
                        qp = q_proj(qt + 1)
                        f1 += [(4, [qp[0]]), (10, [qp[1]])]
                    else:
                        # qt3 pr0 norm pieces run inside block (1,3):
                        # their oraw slots + recips are ready after (0,3)
                        nps3 = norm_pieces(3)
                        f1 += [(12, [nps3[0]]), (14, [nps3[1]])]
                attn_block(0, qt, fillers=f0)
                recip_block(0, qt)
                attn_block(1, qt, fillers=f1)
                recip_block(1, qt)
            # tail: remaining norm (pr1) + out-proj for qt=3
            for fn in nps3[2:]:
                fn()
            for fn in proj_pieces(3, evac_scalar=True):
                fn()

    nc.finalize()
    return nc


_NC_CACHE = None


def _get_nc():
    global _NC_CACHE
    if _NC_CACHE is None:
        _NC_CACHE = build_nc()
    return _NC_CACHE


def _swz(wT):
    """[C*128, cols] -> DMA-contiguous [128, C*cols] (partition-major)."""
    C = wT.shape[0] // 128
    return np.ascontiguousarray(
        wT.reshape(C, 128, -1).swapaxes(0, 1).reshape(128, -1)).astype(bf16)


def _xswz(xT):
    """[D, T] -> [128, NTB, 8, 512] (t-super-block-major)."""
    # element (p, tb, c, t) = xT[c*128+p, tb*512+t]
    return np.ascontiguousarray(
        xT.reshape(8, 128, NTB, 512).transpose(1, 2, 0, 3)).astype(bf16)


def make_in_maps(query, key, value, wq, bq, wk, bk, wv, bv, wo, bo):
    in_maps = []
    for c in range(N_CORES):
        b, hg = divmod(c, HPC)
        sl = slice(hg * W, (hg + 1) * W)
        in_maps.append({
            "xq": _xswz(np.asarray(query[b]).T),
            "xk": _xswz(np.asarray(key[b]).T),
            "xv": _xswz(np.asarray(value[b]).T),
            "wq": _swz(np.asarray(wq)[sl].T),
            "wk": _swz(np.asarray(wk)[sl].T),
            "wv": _swz(np.asarray(wv)[sl].T),
            "wo": _swz(np.asarray(wo)[:, sl].T),
            "bqc": np.ascontiguousarray(
                np.asarray(bq)[sl].reshape(2, 128).T).astype(np.float32),
            "bkc": np.ascontiguousarray(
                np.asarray(bk)[sl].reshape(2, 128).T).astype(np.float32),
            "bv": np.asarray(bv)[sl].reshape(1, W).astype(bf16),
        })
    return in_maps


def combine_outputs(outs, bo):
    full = np.zeros((B, T, D), np.float32)
    for c in range(N_CORES):
        b = c // HPC
        full[b] += outs[c].T.astype(np.float32)
    full += np.asarray(bo, np.float32)[None, None, :]
    return full


def kernel(query, key, value, wq, bq, wk, bk, wv, bv, wo, bo):
    nc = _get_nc()
    in_maps = make_in_maps(query, key, value, wq, bq, wk, bk, wv, bv, wo, bo)
    res = run_bass_kernel_spmd(nc, in_maps, list(range(N_CORES)))
    outs = [np.asarray(res.results[c]["out"]) for c in range(N_CORES)]
    return combine_outputs(outs, bo)
